# revision 2
# baseline (speedup 1.0000x reference)
"""Trainium2 Bass kernel for nn_BoundleAdjustment.

Strategy: observations are data-parallel sharded across the 8 NeuronCores
(M/8 each).  The host stages per-observation records (pose row, patch row,
target, weight) into 14 planar [128, 2048] f32 streams per core; the device
kernel streams them through SBUF and performs the full math on-chip:
quaternion normalization (via 2/s scaling), SE3 rotation+translation
(two cross products), cartesian->polar (sqrt/atan2 with quadrant fixup on
the Scalar engine's Arctan/Sqrt/Square/Sign tables), and weighted residual.

Elementwise work is split between the Vector engine (DVE) and GPSIMD (Pool)
by observation columns to balance engine occupancy; transcendentals run on
the Scalar (ACT) engine.
"""

import numpy as np

M = 2097152
NCORES = 8
N = M // NCORES          # 262144 obs per core
P = 128
COLS = N // P            # 2048 obs per partition
CC = 512                 # obs columns per chunk
NCH = COLS // CC         # 4 chunks
NPLANES = 14             # tx ty tz qx qy qz qw px py pz X Y Z w
PI = float(np.pi)

# fraction of each chunk's columns processed on GPSIMD (Pool) instead of DVE
# Pool fp32 tensor_tensor is ~2.17x slower per element than DVE; giving Pool
# ~27% of the columns roughly equalizes the two engines' busy time.
POOL_COLS = 0

_CACHE = {}


def _emit_compute(nc, mybir, eng, tiles, tmp, lo, w):
    """Emit the per-chunk math for columns [lo, lo+w) using engine `eng`
    (nc.vector or nc.gpsimd) for tensor_tensor/tensor_scalar ops.
    ACT ops always run on nc.scalar; reciprocal always on nc.vector."""
    AF = mybir.ActivationFunctionType
    OP = mybir.AluOpType
    act = nc.scalar
    vec = nc.vector

    def s_(t):
        return t[:, lo:lo + w]

    (tx, ty, tz, qx, qy, qz, qw, px, py, pz, X, Y, Z, W) = tiles

    def tt(dst, a, b, op):
        eng.tensor_tensor(out=s_(dst), in0=s_(a), in1=s_(b), op=op)

    # s = |q|^2 ; u = 2/s
    act.activation(s_(tmp["sqx"]), s_(qx), AF.Square)
    act.activation(s_(tmp["sqy"]), s_(qy), AF.Square)
    act.activation(s_(tmp["sqz"]), s_(qz), AF.Square)
    act.activation(s_(tmp["sqw"]), s_(qw), AF.Square)
    tt(tmp["s01"], tmp["sqx"], tmp["sqy"], OP.add)
    tt(tmp["s23"], tmp["sqz"], tmp["sqw"], OP.add)
    tt(tmp["s"], tmp["s01"], tmp["s23"], OP.add)
    eng.tensor_scalar(out=s_(tmp["s"]), in0=s_(tmp["s"]), scalar1=0.5,
                      scalar2=None, op0=OP.mult)
    vec.reciprocal(s_(tmp["u"]), s_(tmp["s"]))          # u = 2/|q|^2

    # uv = qv x pts
    tt(tmp["uvx"], qy, pz, OP.mult)
    tt(tmp["t0"], qz, py, OP.mult)
    tt(tmp["uvx"], tmp["uvx"], tmp["t0"], OP.subtract)
    tt(tmp["uvy"], qz, px, OP.mult)
    tt(tmp["t1"], qx, pz, OP.mult)
    tt(tmp["uvy"], tmp["uvy"], tmp["t1"], OP.subtract)
    tt(tmp["uvz"], qx, py, OP.mult)
    tt(tmp["t2"], qy, px, OP.mult)
    tt(tmp["uvz"], tmp["uvz"], tmp["t2"], OP.subtract)

    # wv = qw*uv + qv x uv
    tt(tmp["wx"], qw, tmp["uvx"], OP.mult)
    tt(tmp["t0"], qy, tmp["uvz"], OP.mult)
    tt(tmp["wx"], tmp["wx"], tmp["t0"], OP.add)
    tt(tmp["t1"], qz, tmp["uvy"], OP.mult)
    tt(tmp["wx"], tmp["wx"], tmp["t1"], OP.subtract)

    tt(tmp["wy"], qw, tmp["uvy"], OP.mult)
    tt(tmp["t0"], qz, tmp["uvx"], OP.mult)
    tt(tmp["wy"], tmp["wy"], tmp["t0"], OP.add)
    tt(tmp["t1"], qx, tmp["uvz"], OP.mult)
    tt(tmp["wy"], tmp["wy"], tmp["t1"], OP.subtract)

    tt(tmp["wz"], qw, tmp["uvz"], OP.mult)
    tt(tmp["t0"], qx, tmp["uvy"], OP.mult)
    tt(tmp["wz"], tmp["wz"], tmp["t0"], OP.add)
    tt(tmp["t1"], qy, tmp["uvx"], OP.mult)
    tt(tmp["wz"], tmp["wz"], tmp["t1"], OP.subtract)

    # r = pts + u*wv + t
    for (wc, pc, tc_, rc) in (("wx", px, tx, "rx"), ("wy", py, ty, "ry"),
                              ("wz", pz, tz, "rz")):
        tt(tmp["t0"], tmp[wc], tmp["u"], OP.mult)
        tt(tmp["t0"], tmp["t0"], pc, OP.add)
        tt(tmp[rc], tmp["t0"], tc_, OP.add)

    # polar
    act.activation(s_(tmp["t0"]), s_(tmp["rx"]), AF.Square)
    act.activation(s_(tmp["t1"]), s_(tmp["ry"]), AF.Square)
    tt(tmp["rho2"], tmp["t0"], tmp["t1"], OP.add)
    act.activation(s_(tmp["t2"]), s_(tmp["rz"]), AF.Square)
    tt(tmp["r2"], tmp["rho2"], tmp["t2"], OP.add)
    act.activation(s_(tmp["rng"]), s_(tmp["r2"]), AF.Sqrt)
    act.activation(s_(tmp["rho"]), s_(tmp["rho2"]), AF.Sqrt)

    # az = atan(ry/rx) + sign(ry)*pi*(rx<0)
    vec.reciprocal(s_(tmp["t0"]), s_(tmp["rx"]))
    tt(tmp["t1"], tmp["ry"], tmp["t0"], OP.mult)
    act.activation(s_(tmp["az"]), s_(tmp["t1"]), AF.Arctan)
    eng.tensor_scalar(out=s_(tmp["t2"]), in0=s_(tmp["rx"]), scalar1=0.0,
                      scalar2=PI, op0=OP.is_lt, op1=OP.mult)
    act.activation(s_(tmp["t0"]), s_(tmp["ry"]), AF.Sign)
    tt(tmp["t2"], tmp["t2"], tmp["t0"], OP.mult)
    tt(tmp["az"], tmp["az"], tmp["t2"], OP.add)

    # el = atan(rz/rho)
    vec.reciprocal(s_(tmp["t0"]), s_(tmp["rho"]))
    tt(tmp["t1"], tmp["rz"], tmp["t0"], OP.mult)
    act.activation(s_(tmp["el"]), s_(tmp["t1"]), AF.Arctan)

    # residual = (proj - target) * w
    for (pc, tgt, oc) in (("rng", X, "o0"), ("az", Y, "o1"), ("el", Z, "o2")):
        tt(tmp[oc], tmp[pc], tgt, OP.subtract)
        tt(tmp[oc], tmp[oc], W, OP.mult)


def _build():
    import concourse.bass as bass
    import concourse.tile as tile
    from concourse import bacc, mybir

    nc = bacc.Bacc("TRN2", target_bir_lowering=False, debug=False,
                   num_devices=NCORES)
    f32 = mybir.dt.float32
    in_d = nc.declare_dram_parameter("in", [NPLANES, P, COLS], f32,
                                     isOutput=False)
    out_d = nc.declare_dram_parameter("out", [3, P, COLS], f32, isOutput=True)

    TMPNAMES = ["sqx", "sqy", "sqz", "sqw", "s01", "s23", "s", "u",
                "uvx", "uvy", "uvz", "t0", "t1", "t2",
                "wx", "wy", "wz", "rx", "ry", "rz",
                "rho2", "r2", "rng", "rho", "az", "el", "o0", "o1", "o2"]

    with tile.TileContext(nc) as tc:
        with tc.tile_pool(name="inp", bufs=2) as inp, \
             tc.tile_pool(name="tmpp", bufs=2) as tmpp, \
             tc.tile_pool(name="outp", bufs=2) as outp:
            for ch in range(NCH):
                tiles = []
                for k in range(NPLANES):
                    t = inp.tile([P, CC], f32, tag=f"in{k}", name=f"in{k}_{ch}")
                    nc.gpsimd.dma_start(t[:], in_d[k, :, bass.ts(ch, CC)])
                    tiles.append(t)
                tmp = {nm: tmpp.tile([P, CC], f32, tag=nm, name=f"tmp_{nm}_{ch}")
                       for nm in TMPNAMES}
                if POOL_COLS:
                    _emit_compute(nc, mybir, nc.vector, tiles, tmp, 0,
                                  CC - POOL_COLS)
                    _emit_compute(nc, mybir, nc.gpsimd, tiles, tmp,
                                  CC - POOL_COLS, POOL_COLS)
                else:
                    _emit_compute(nc, mybir, nc.vector, tiles, tmp, 0, CC)
                for i, nm in enumerate(("o0", "o1", "o2")):
                    nc.gpsimd.dma_start(out_d[i, :, bass.ts(ch, CC)],
                                        tmp[nm][:])
    nc.finalize()
    return nc


def _get_nc():
    if "nc" not in _CACHE:
        _CACHE["nc"] = _build()
    return _CACHE["nc"]


def kernel(poses, patch_coords, elevation_angle, poses_idx, patch_idx,
           target_coords, weights):
    from concourse.bass_utils import run_bass_kernel_spmd

    poses = np.asarray(poses, dtype=np.float32)
    patch_coords = np.asarray(patch_coords, dtype=np.float32)
    elevation_angle = np.asarray(elevation_angle, dtype=np.float32)
    target_coords = np.asarray(target_coords, dtype=np.float32)
    weights = np.asarray(weights, dtype=np.float32)
    pid = np.asarray(poses_idx).astype(np.int64)
    qid = np.asarray(patch_idx).astype(np.int64)

    # host-side staging: shard observations across cores and lay the
    # per-observation records out as 14 planar [128, COLS] streams per core.
    p7 = poses[pid]                                           # [M, 7]
    pts = np.concatenate(
        [patch_coords[qid], elevation_angle[qid]], axis=1)    # [M, 3]
    big = np.concatenate([p7, pts, target_coords, weights], axis=1)  # [M,14]
    big = np.ascontiguousarray(
        big.reshape(NCORES, P, COLS, NPLANES).transpose(0, 3, 1, 2))

    nc = _get_nc()
    in_maps = [{"in": big[c]} for c in range(NCORES)]
    res = run_bass_kernel_spmd(nc, in_maps, list(range(NCORES)))
    out = np.stack([res.results[c]["out"] for c in range(NCORES)])
    # [NCORES, 3, P, COLS] -> [M, 3]
    return np.ascontiguousarray(
        out.transpose(0, 2, 3, 1).reshape(M, 3)).astype(np.float32)
